# revision 9
# baseline (speedup 1.0000x reference)
"""CAM (channel attention) module kernel for Trainium2, 8-core data-parallel.

Reference computation (per sample b):
    q = conv2d(x, Wq, stride2, 2x2) -> [C, 4096]
    k = conv2d(x, Wk, stride2, 2x2) -> [C, 4096]
    v = conv2d(x, Wv, 1x1)          -> [C, 16384]
    E = q @ k^T                      [C, C]
    att = softmax(rowmax(E) - E)   (== softmin over rows)
    out = att @ v -> [C, H, W]

Kernel strategy (one sample per NeuronCore, B=8 over 8 cores):
  - The softmax here is extremely peaked (energy entries span +-200), so
    energy errors are amplified exponentially: q/k need ~18+ mantissa
    bits, which rules out bf16 and single-pass fp32r (12-bit) for the
    convs. Native fp32 matmul costs 2 half-rate passes (4 cyc/row).
  - Instead: split-fp32r. x = xr + xl2 with xr = round_f32r(x) and
    xl2 = round_f32r(x - xr) (~24-bit combined); same for the conv
    weights. conv = Wr@xr + Wr@xl2 + Wl2@xr: 3 full-rate passes
    (3 cyc/row) at fp32-level accuracy (dropped Wl2@xl2 term is
    ~2^-24 relative).
  - conv produces q in [c, n] layout via strided im2col APs from the
    resident xr/xl2 tiles (4 accumulating taps x 3 passes per band),
    then PE-transposes to [n, c] chunks for the energy contraction.
  - energy e = q k^T in native fp32 (exact; N=128 makes fp32r slow
    there anyway), accumulated over 32 chunk matmuls in one PSUM bank.
  - softmax via one DVE row-min + one ScalarE exp (bias=rowmin,
    scale=-1) with fused accumulated row-sum.
  - out = att @ (Wv x + bv) == (att Wv) @ x + (att bv) 1^T: computes
    M^T = Wv^T att^T on PE ([128,128]), then out = M @ xr as single-pass
    fp32r matmuls against the already-rounded xr tiles (contraction is
    only 128 deep and feeds no exponential; ~1.5e-4 worst-case impact).
"""

import numpy as np

B, C, H, W = 8, 128, 128, 128
HW = H * W           # 16384
N_CORES = 8
NB = 8               # number of H-bands (16 input rows each)
BAND = HW // NB      # 2048 x columns per band
QN = (H // 2) * (W // 2)  # 4096 conv output positions
QCHUNK = QN // NB    # 512 conv outputs per band

_CACHE = {}


def _build_program(with_qk_bias: bool, with_v_bias: bool):
    import concourse.tile as tile
    from concourse import bacc, mybir
    from concourse.masks import make_identity

    f32 = mybir.dt.float32
    f32r = mybir.dt.float32r
    Ident = mybir.ActivationFunctionType.Identity
    CopyF = mybir.ActivationFunctionType.Copy
    nc = bacc.Bacc(
        "TRN2", target_bir_lowering=False, debug=False, num_devices=N_CORES)

    x_d = nc.declare_dram_parameter("x", [C, HW], f32, isOutput=False)
    wqT_d = nc.declare_dram_parameter("wqT", [C, 4 * C], f32, isOutput=False)
    wkT_d = nc.declare_dram_parameter("wkT", [C, 4 * C], f32, isOutput=False)
    wv_d = nc.declare_dram_parameter("wv", [C, C], f32, isOutput=False)
    if with_qk_bias:
        bq_d = nc.declare_dram_parameter("bq", [C, 1], f32, isOutput=False)
        bk_d = nc.declare_dram_parameter("bk", [C, 1], f32, isOutput=False)
    if with_v_bias:
        bv_d = nc.declare_dram_parameter("bv", [C, 1], f32, isOutput=False)
    out_d = nc.declare_dram_parameter("out", [C, HW], f32, isOutput=True)

    with tile.TileContext(nc) as tc:
        with (
            tc.tile_pool(name="const", bufs=1) as const,
            tc.tile_pool(name="xstage", bufs=3) as xstage,
            tc.tile_pool(name="xrp", bufs=1) as xrp,
            tc.tile_pool(name="xl2p", bufs=3) as xl2p,
            tc.tile_pool(name="qkT", bufs=1) as qkT,
            tc.tile_pool(name="stage", bufs=3) as stage,
            tc.tile_pool(name="oout", bufs=2) as oout,
            tc.tile_pool(name="small", bufs=2) as small,
            tc.tile_pool(name="pacc", bufs=3, space="PSUM") as pacc,
            tc.tile_pool(name="ptp", bufs=2, space="PSUM") as ptp,
            tc.tile_pool(name="psm", bufs=1, space="PSUM") as psm,
        ):
            ident = const.tile([128, 128], f32, tag="ident")
            make_identity(nc, ident)

            # DMA order: x band 0 first (first conv matmul's critical path),
            # then weights, then the rest of x.
            x_sb = []
            xr_sb = [xrp.tile([C, BAND], f32r, tag=f"xr{j}", name=f"xr{j}")
                     for j in range(NB)]
            x0 = xstage.tile([C, BAND], f32, tag="x")
            nc.sync.dma_start(out=x0, in_=x_d[:, 0:BAND])
            x_sb.append(x0)
            wqT_sb = const.tile([C, 4 * C], f32, tag="wqT")
            nc.sync.dma_start(out=wqT_sb, in_=wqT_d[:, :])
            wkT_sb = const.tile([C, 4 * C], f32, tag="wkT")
            nc.sync.dma_start(out=wkT_sb, in_=wkT_d[:, :])
            if with_qk_bias:
                bq_sb = const.tile([C, 1], f32, tag="bq")
                nc.sync.dma_start(out=bq_sb, in_=bq_d[:, :])
                bk_sb = const.tile([C, 1], f32, tag="bk")
                nc.sync.dma_start(out=bk_sb, in_=bk_d[:, :])
            for j in range(1, NB):
                t = xstage.tile([C, BAND], f32, tag="x", name=f"x{j}")
                nc.sync.dma_start(out=t, in_=x_d[:, j * BAND:(j + 1) * BAND])
                x_sb.append(t)
            wv_sb = const.tile([C, C], f32, tag="wv")
            nc.sync.dma_start(out=wv_sb, in_=wv_d[:, :])
            if with_v_bias:
                bv_sb = const.tile([C, 1], f32, tag="bv")
                nc.sync.dma_start(out=bv_sb, in_=bv_d[:, :])

            # split the conv weights into f32r hi/lo parts on DVE (tiny)
            def split_w(w_f32, name):
                hi = const.tile([C, 4 * C], f32r, tag=f"{name}hi")
                nc.vector.tensor_copy(hi, w_f32)
                lo = const.tile([C, 4 * C], f32r, tag=f"{name}lo")
                nc.vector.tensor_tensor(
                    out=lo, in0=w_f32, in1=hi[:].bitcast(f32),
                    op=mybir.AluOpType.subtract)
                return hi, lo

            wqh, wql = split_w(wqT_sb, "wq")
            wkh, wkl = split_w(wkT_sb, "wk")

            qT = [qkT.tile([128, QCHUNK], f32, tag=f"qT{j}", name=f"qT{j}")
                  for j in range(NB)]
            kT = [qkT.tile([128, QCHUNK], f32, tag=f"kT{j}", name=f"kT{j}")
                  for j in range(NB)]

            def conv_band(j, wh, wl, xr_v, xl_v, bias_sb):
                """12 accumulating matmuls -> PSUM [128, 512] (q for band j),
                returns the psum tile."""
                acc = pacc.tile([128, QCHUNK], f32, tag="acc")
                n_mm = 0
                for ab in range(4):
                    a, bb = ab // 2, ab % 2
                    for lhsT, rhs in (
                        (wh[:, ab * C:(ab + 1) * C], xr_v[:, :, a, :, bb]),
                        (wh[:, ab * C:(ab + 1) * C], xl_v[:, :, a, :, bb]),
                        (wl[:, ab * C:(ab + 1) * C], xr_v[:, :, a, :, bb]),
                    ):
                        nc.tensor.matmul(acc, lhsT=lhsT, rhs=rhs,
                                         start=(n_mm == 0), stop=(n_mm == 11))
                        n_mm += 1
                return acc

            def emit_transposes(j, qc, kc):
                for T_out, src in ((qT[j], qc), (kT[j], kc)):
                    tp = ptp.tile([128, QCHUNK], f32, tag="tp")
                    for t in range(4):
                        nc.tensor.transpose(
                            tp[:, t * 128:(t + 1) * 128],
                            src[:, t * 128:(t + 1) * 128], ident)
                    nc.scalar.activation(out=T_out, in_=tp, func=CopyF,
                                         bias=0.0, scale=1.0)

            pend = None
            for j in range(NB):
                # rounding passes for band j (DVE)
                xr_t = xr_sb[j]
                nc.vector.tensor_copy(xr_t, x_sb[j])
                xl_t = xl2p.tile([C, BAND], f32r, tag="xl2", name=f"xl{j}")
                nc.vector.tensor_tensor(
                    out=xl_t, in0=x_sb[j], in1=xr_t[:].bitcast(f32),
                    op=mybir.AluOpType.subtract)
                xr_v = xr_t[:].rearrange(
                    "p (i a w b) -> p i a w b", i=8, a=2, w=64, b=2)
                xl_v = xl_t[:].rearrange(
                    "p (i a w b) -> p i a w b", i=8, a=2, w=64, b=2)

                acc_q = conv_band(j, wqh, wql, xr_v, xl_v, None)
                acc_k = conv_band(j, wkh, wkl, xr_v, xl_v, None)

                # PSUM -> SBUF chunk copies (+ conv bias) on ScalarE
                qc = stage.tile([128, QCHUNK], f32, tag="qchunk")
                kc = stage.tile([128, QCHUNK], f32, tag="kchunk")
                if with_qk_bias:
                    nc.scalar.activation(out=qc, in_=acc_q, func=Ident,
                                         bias=bq_sb[:, 0:1], scale=1.0)
                    nc.scalar.activation(out=kc, in_=acc_k, func=Ident,
                                         bias=bk_sb[:, 0:1], scale=1.0)
                else:
                    nc.scalar.activation(out=qc, in_=acc_q, func=CopyF,
                                         bias=0.0, scale=1.0)
                    nc.scalar.activation(out=kc, in_=acc_k, func=CopyF,
                                         bias=0.0, scale=1.0)
                # transposes one band behind so PE never stalls on the copies
                if pend is not None:
                    emit_transposes(*pend)
                pend = (j, qc, kc)
            emit_transposes(*pend)

            # energy E[c, d] over all 32 n-chunks, native fp32 (exact)
            E = psm.tile([128, 128], f32, tag="E")
            idx = 0
            for j in range(NB):
                for t in range(4):
                    nc.tensor.matmul(
                        E,
                        lhsT=qT[j][:, t * 128:(t + 1) * 128],
                        rhs=kT[j][:, t * 128:(t + 1) * 128],
                        start=(idx == 0), stop=(idx == NB * 4 - 1))
                    idx += 1

            # softmin over rows: att = exp(rowmin - E) / Z
            mmin = small.tile([128, 1], f32, tag="mmin")
            nc.vector.tensor_reduce(
                out=mmin, in_=E, axis=mybir.AxisListType.X,
                op=mybir.AluOpType.min)
            w_sb = small.tile([128, 128], f32, tag="w")
            zsum = small.tile([128, 1], f32, tag="z")
            nc.scalar.activation(
                out=w_sb, in_=E, func=mybir.ActivationFunctionType.Exp,
                bias=mmin[:, 0:1], scale=-1.0, accum_out=zsum[:, 0:1])
            rz = small.tile([128, 1], f32, tag="rz")
            nc.vector.reciprocal(rz, zsum)
            att = small.tile([128, 128], f32, tag="att")
            nc.vector.tensor_scalar_mul(att, w_sb, rz[:, 0:1])

            attT_p = psm.tile([128, 128], f32, tag="s2")
            nc.tensor.transpose(attT_p, att, ident)
            attT = small.tile([128, 128], f32, tag="attT")
            nc.vector.tensor_copy(attT, attT_p)

            # M^T[c2, c] = sum_d Wv[d, c2] attT[d, c], rounded to f32r
            MT_p = psm.tile([128, 128], f32, tag="s2")
            nc.tensor.matmul(MT_p, lhsT=wv_sb, rhs=attT, start=True, stop=True)
            MT = small.tile([128, 128], f32r, tag="MT")
            nc.vector.tensor_copy(MT, MT_p)

            if with_v_bias:
                abv_p = psm.tile([128, 1], f32, tag="s2")
                nc.tensor.matmul(abv_p, lhsT=attT, rhs=bv_sb[:, 0:1],
                                 start=True, stop=True)
                abv = small.tile([128, 1], f32, tag="abv")
                nc.vector.tensor_copy(abv, abv_p)

            # out[c, n] = sum_c2 M[c, c2] xr[c2, n] (+ abv[c]), fp32r
            for j in range(NB):
                o_band = oout.tile([128, BAND], f32, tag="oband")
                for s in range(BAND // 512):
                    o_p = pacc.tile([128, 512], f32, tag="acc")
                    nc.tensor.matmul(
                        o_p, lhsT=MT,
                        rhs=xr_sb[j][:, s * 512:(s + 1) * 512],
                        start=True, stop=True)
                    dst = o_band[:, s * 512:(s + 1) * 512]
                    if with_v_bias:
                        nc.scalar.activation(
                            out=dst, in_=o_p, func=Ident,
                            bias=abv[:, 0:1], scale=1.0)
                    elif s % 2 == 0:
                        nc.vector.tensor_copy(dst, o_p)
                    else:
                        nc.scalar.activation(out=dst, in_=o_p, func=CopyF,
                                             bias=0.0, scale=1.0)
                nc.sync.dma_start(
                    out=out_d[:, j * BAND:(j + 1) * BAND], in_=o_band)

    nc.compile()
    return nc


def kernel(x, Wq, bq, Wk, bk, Wv, bv):
    from concourse.bass_utils import run_bass_kernel_spmd

    x = np.ascontiguousarray(np.asarray(x, dtype=np.float32))
    Wq = np.asarray(Wq, dtype=np.float32)
    Wk = np.asarray(Wk, dtype=np.float32)
    Wv = np.asarray(Wv, dtype=np.float32)
    bq = np.asarray(bq, dtype=np.float32)
    bk = np.asarray(bk, dtype=np.float32)
    bv = np.asarray(bv, dtype=np.float32)

    with_qk_bias = bool(np.any(bq) or np.any(bk))
    with_v_bias = bool(np.any(bv))

    key = (with_qk_bias, with_v_bias)
    if key not in _CACHE:
        _CACHE[key] = _build_program(with_qk_bias, with_v_bias)
    nc = _CACHE[key]

    # weight layout prep: wT[cin, ab*128 + c] = W[c, cin, a, b]
    wqT = np.ascontiguousarray(Wq.transpose(1, 2, 3, 0).reshape(C, 4 * C))
    wkT = np.ascontiguousarray(Wk.transpose(1, 2, 3, 0).reshape(C, 4 * C))
    wv = np.ascontiguousarray(Wv.reshape(C, C))

    in_maps = []
    for b in range(B):
        m = {
            "x": np.ascontiguousarray(x[b].reshape(C, HW)),
            "wqT": wqT,
            "wkT": wkT,
            "wv": wv,
        }
        if with_qk_bias:
            m["bq"] = np.ascontiguousarray(bq.reshape(C, 1))
            m["bk"] = np.ascontiguousarray(bk.reshape(C, 1))
        if with_v_bias:
            m["bv"] = np.ascontiguousarray(bv.reshape(C, 1))
        in_maps.append(m)

    res = run_bass_kernel_spmd(nc, in_maps, list(range(N_CORES)))
    out = np.stack([res.results[i]["out"] for i in range(N_CORES)])
    return out.reshape(B, C, H, W).astype(np.float32)


# revision 11
# speedup vs baseline: 1.2695x; 1.2695x over previous
"""CAM (channel attention) module kernel for Trainium2, 8-core data-parallel.

Reference computation (per sample b):
    q = conv2d(x, Wq, stride2, 2x2) -> [C, 4096]
    k = conv2d(x, Wk, stride2, 2x2) -> [C, 4096]
    v = conv2d(x, Wv, 1x1)          -> [C, 16384]
    E = q @ k^T                      [C, C]
    att = softmax(rowmax(E) - E)   (== softmin over rows)
    out = att @ v -> [C, H, W]

Kernel strategy (one sample per NeuronCore, B=8 over 8 cores):
  - The softmax here is extremely peaked (energy entries span +-200), so
    energy errors are amplified exponentially: q/k need ~18+ mantissa
    bits, which rules out bf16 and single-pass fp32r (12-bit) for the
    convs. Native fp32 matmul costs 2 half-rate passes (4 cyc/row).
  - Measured on HW: 4-byte moving operands (fp32 AND fp32r) stream at
    2 cyc/row; bf16 streams at 1 cyc/row. So the cheapest precise
    scheme is split-bf16: x = xh + xl with xh = bf16(x), xl =
    bf16(x - xh) (~16-bit combined); same for the conv weights.
    conv = Wh@xh + Wh@xl + Wl@xh: 3 full-rate bf16 passes (3 cyc/row
    vs fp32's 4) with ~6.5e-4 worst-case output impact.
  - conv produces q in [c, n] layout via strided im2col APs from the
    resident xr/xl2 tiles (4 accumulating taps x 3 passes per band),
    then PE-transposes to [n, c] chunks for the energy contraction.
  - energy e = q k^T in native fp32 (exact; N=128 makes fp32r slow
    there anyway), accumulated over 32 chunk matmuls in one PSUM bank.
  - softmax via one DVE row-min + one ScalarE exp (bias=rowmin,
    scale=-1) with fused accumulated row-sum.
  - out = att @ (Wv x + bv) == (att Wv) @ x + (att bv) 1^T: computes
    M^T = Wv^T att^T on PE ([128,128]), splits M the same way, and
    runs out = Mh@xh + Mh@xl + Ml@xh against the resident split-x
    tiles (3 bf16 passes, ~1e-5 error; reuses the conv's xh/xl).
"""

import numpy as np

B, C, H, W = 8, 128, 128, 128
HW = H * W           # 16384
N_CORES = 8
NB = 8               # number of H-bands (16 input rows each)
BAND = HW // NB      # 2048 x columns per band
QN = (H // 2) * (W // 2)  # 4096 conv output positions
QCHUNK = QN // NB    # 512 conv outputs per band

_CACHE = {}


def _build_program(with_qk_bias: bool, with_v_bias: bool):
    import concourse.tile as tile
    from concourse import bacc, mybir
    from concourse.masks import make_identity

    f32 = mybir.dt.float32
    bf16 = mybir.dt.bfloat16
    Ident = mybir.ActivationFunctionType.Identity
    CopyF = mybir.ActivationFunctionType.Copy
    nc = bacc.Bacc(
        "TRN2", target_bir_lowering=False, debug=False, num_devices=N_CORES)

    x_d = nc.declare_dram_parameter("x", [C, HW], f32, isOutput=False)
    wqT_d = nc.declare_dram_parameter("wqT", [C, 4 * C], f32, isOutput=False)
    wkT_d = nc.declare_dram_parameter("wkT", [C, 4 * C], f32, isOutput=False)
    wv_d = nc.declare_dram_parameter("wv", [C, C], f32, isOutput=False)
    if with_qk_bias:
        bq_d = nc.declare_dram_parameter("bq", [C, 1], f32, isOutput=False)
        bk_d = nc.declare_dram_parameter("bk", [C, 1], f32, isOutput=False)
    if with_v_bias:
        bv_d = nc.declare_dram_parameter("bv", [C, 1], f32, isOutput=False)
    out_d = nc.declare_dram_parameter("out", [C, HW], f32, isOutput=True)

    with tile.TileContext(nc) as tc:
        with (
            tc.tile_pool(name="const", bufs=1) as const,
            tc.tile_pool(name="xstage", bufs=3) as xstage,
            tc.tile_pool(name="xrp", bufs=1) as xrp,
            tc.tile_pool(name="xl2p", bufs=3) as xl2p,
            tc.tile_pool(name="qkT", bufs=1) as qkT,
            tc.tile_pool(name="stage", bufs=3) as stage,
            tc.tile_pool(name="oout", bufs=2) as oout,
            tc.tile_pool(name="small", bufs=2) as small,
            tc.tile_pool(name="pacc", bufs=3, space="PSUM") as pacc,
            tc.tile_pool(name="ptp", bufs=2, space="PSUM") as ptp,
            tc.tile_pool(name="psm", bufs=1, space="PSUM") as psm,
        ):
            ident = const.tile([128, 128], f32, tag="ident")
            make_identity(nc, ident)

            # DMA order: x band 0 first (first conv matmul's critical path),
            # then weights, then the rest of x.
            x_sb = []
            xh_sb = [xrp.tile([C, BAND], bf16, tag=f"xh{j}", name=f"xh{j}")
                     for j in range(NB)]
            xl_sb = [xrp.tile([C, BAND], bf16, tag=f"xl{j}", name=f"xl{j}")
                     for j in range(NB)]
            x0 = xstage.tile([C, BAND], f32, tag="x")
            nc.sync.dma_start(out=x0, in_=x_d[:, 0:BAND])
            x_sb.append(x0)
            wqT_sb = const.tile([C, 4 * C], f32, tag="wqT")
            nc.sync.dma_start(out=wqT_sb, in_=wqT_d[:, :])
            wkT_sb = const.tile([C, 4 * C], f32, tag="wkT")
            nc.sync.dma_start(out=wkT_sb, in_=wkT_d[:, :])
            if with_qk_bias:
                bq_sb = const.tile([C, 1], f32, tag="bq")
                nc.sync.dma_start(out=bq_sb, in_=bq_d[:, :])
                bk_sb = const.tile([C, 1], f32, tag="bk")
                nc.sync.dma_start(out=bk_sb, in_=bk_d[:, :])
            for j in range(1, NB):
                t = xstage.tile([C, BAND], f32, tag="x", name=f"x{j}")
                eng = nc.sync if j % 2 == 0 else nc.gpsimd
                eng.dma_start(out=t, in_=x_d[:, j * BAND:(j + 1) * BAND])
                x_sb.append(t)
            wv_sb = const.tile([C, C], f32, tag="wv")
            nc.sync.dma_start(out=wv_sb, in_=wv_d[:, :])
            if with_v_bias:
                bv_sb = const.tile([C, 1], f32, tag="bv")
                nc.sync.dma_start(out=bv_sb, in_=bv_d[:, :])

            # split the conv weights into bf16 hi/lo parts on DVE (tiny)
            def split_w(w_f32, name):
                hi = const.tile([C, 4 * C], bf16, tag=f"{name}hi")
                nc.vector.tensor_copy(hi, w_f32)
                lo = const.tile([C, 4 * C], bf16, tag=f"{name}lo")
                nc.vector.tensor_tensor(
                    out=lo, in0=w_f32, in1=hi,
                    op=mybir.AluOpType.subtract)
                return hi, lo

            wqh, wql = split_w(wqT_sb, "wq")
            wkh, wkl = split_w(wkT_sb, "wk")

            qT = [qkT.tile([128, QCHUNK], f32, tag=f"qT{j}", name=f"qT{j}")
                  for j in range(NB)]
            kT = [qkT.tile([128, QCHUNK], f32, tag=f"kT{j}", name=f"kT{j}")
                  for j in range(NB)]

            def conv_band(j, wh, wl, xr_v, xl_v, bias_sb):
                """12 accumulating matmuls -> PSUM [128, 512] (q for band j),
                returns the psum tile."""
                acc = pacc.tile([128, QCHUNK], f32, tag="acc")
                n_mm = 0
                for ab in range(4):
                    a, bb = ab // 2, ab % 2
                    for lhsT, rhs in (
                        (wh[:, ab * C:(ab + 1) * C], xr_v[:, :, a, :, bb]),
                        (wh[:, ab * C:(ab + 1) * C], xl_v[:, :, a, :, bb]),
                        (wl[:, ab * C:(ab + 1) * C], xr_v[:, :, a, :, bb]),
                    ):
                        nc.tensor.matmul(acc, lhsT=lhsT, rhs=rhs,
                                         start=(n_mm == 0), stop=(n_mm == 11))
                        n_mm += 1
                return acc

            def emit_transposes(j, qc, kc):
                for T_out, src in ((qT[j], qc), (kT[j], kc)):
                    tp = ptp.tile([128, QCHUNK], f32, tag="tp")
                    for t in range(4):
                        nc.tensor.transpose(
                            tp[:, t * 128:(t + 1) * 128],
                            src[:, t * 128:(t + 1) * 128], ident)
                    nc.scalar.activation(out=T_out, in_=tp, func=CopyF,
                                         bias=0.0, scale=1.0)

            pend = None
            for j in range(NB):
                # bf16 hi/lo split for band j (DVE)
                xh_t, xl_t = xh_sb[j], xl_sb[j]
                nc.vector.tensor_copy(xh_t, x_sb[j])
                nc.vector.tensor_tensor(
                    out=xl_t, in0=x_sb[j], in1=xh_t,
                    op=mybir.AluOpType.subtract)
                xr_v = xh_t[:].rearrange(
                    "p (i a w b) -> p i a w b", i=8, a=2, w=64, b=2)
                xl_v = xl_t[:].rearrange(
                    "p (i a w b) -> p i a w b", i=8, a=2, w=64, b=2)

                acc_q = conv_band(j, wqh, wql, xr_v, xl_v, None)
                acc_k = conv_band(j, wkh, wkl, xr_v, xl_v, None)

                # PSUM -> SBUF chunk copies (+ conv bias) on ScalarE
                qc = stage.tile([128, QCHUNK], f32, tag="qchunk")
                kc = stage.tile([128, QCHUNK], f32, tag="kchunk")
                if with_qk_bias:
                    nc.scalar.activation(out=qc, in_=acc_q, func=Ident,
                                         bias=bq_sb[:, 0:1], scale=1.0)
                    nc.scalar.activation(out=kc, in_=acc_k, func=Ident,
                                         bias=bk_sb[:, 0:1], scale=1.0)
                else:
                    nc.scalar.activation(out=qc, in_=acc_q, func=CopyF,
                                         bias=0.0, scale=1.0)
                    nc.scalar.activation(out=kc, in_=acc_k, func=CopyF,
                                         bias=0.0, scale=1.0)
                # transposes one band behind so PE never stalls on the copies
                if pend is not None:
                    emit_transposes(*pend)
                pend = (j, qc, kc)
            emit_transposes(*pend)

            # energy E[c, d] over all 32 n-chunks, native fp32 (exact)
            E = psm.tile([128, 128], f32, tag="E")
            idx = 0
            for j in range(NB):
                for t in range(4):
                    nc.tensor.matmul(
                        E,
                        lhsT=qT[j][:, t * 128:(t + 1) * 128],
                        rhs=kT[j][:, t * 128:(t + 1) * 128],
                        start=(idx == 0), stop=(idx == NB * 4 - 1))
                    idx += 1

            # softmin over rows: att = exp(rowmin - E) / Z
            mmin = small.tile([128, 1], f32, tag="mmin")
            nc.vector.tensor_reduce(
                out=mmin, in_=E, axis=mybir.AxisListType.X,
                op=mybir.AluOpType.min)
            w_sb = small.tile([128, 128], f32, tag="w")
            zsum = small.tile([128, 1], f32, tag="z")
            nc.scalar.activation(
                out=w_sb, in_=E, func=mybir.ActivationFunctionType.Exp,
                bias=mmin[:, 0:1], scale=-1.0, accum_out=zsum[:, 0:1])
            rz = small.tile([128, 1], f32, tag="rz")
            nc.vector.reciprocal(rz, zsum)
            att = small.tile([128, 128], f32, tag="att")
            nc.vector.tensor_scalar_mul(att, w_sb, rz[:, 0:1])

            attT_p = psm.tile([128, 128], f32, tag="s2")
            nc.tensor.transpose(attT_p, att, ident)
            attT = small.tile([128, 128], f32, tag="attT")
            nc.vector.tensor_copy(attT, attT_p)

            # M^T[c2, c] = sum_d Wv[d, c2] attT[d, c], split into bf16 hi/lo
            MT_p = psm.tile([128, 128], f32, tag="s2")
            nc.tensor.matmul(MT_p, lhsT=wv_sb, rhs=attT, start=True, stop=True)
            Mh = small.tile([128, 128], bf16, tag="Mh")
            nc.vector.tensor_copy(Mh, MT_p)
            Ml = small.tile([128, 128], bf16, tag="Ml")
            nc.vector.tensor_tensor(
                out=Ml, in0=MT_p, in1=Mh, op=mybir.AluOpType.subtract)

            if with_v_bias:
                abv_p = psm.tile([128, 1], f32, tag="s2")
                nc.tensor.matmul(abv_p, lhsT=attT, rhs=bv_sb[:, 0:1],
                                 start=True, stop=True)
                abv = small.tile([128, 1], f32, tag="abv")
                nc.vector.tensor_copy(abv, abv_p)

            # out[c, n] = sum_c2 M[c, c2] x[c2, n] (+ abv[c]) via bf16 split
            for j in range(NB):
                o_band = oout.tile([128, BAND], f32, tag="oband")
                for s in range(BAND // 512):
                    o_p = pacc.tile([128, 512], f32, tag="acc")
                    sl = slice(s * 512, (s + 1) * 512)
                    nc.tensor.matmul(o_p, lhsT=Mh, rhs=xh_sb[j][:, sl],
                                     start=True, stop=False)
                    nc.tensor.matmul(o_p, lhsT=Mh, rhs=xl_sb[j][:, sl],
                                     start=False, stop=False)
                    nc.tensor.matmul(o_p, lhsT=Ml, rhs=xh_sb[j][:, sl],
                                     start=False, stop=True)
                    dst = o_band[:, sl]
                    if with_v_bias:
                        nc.scalar.activation(
                            out=dst, in_=o_p, func=Ident,
                            bias=abv[:, 0:1], scale=1.0)
                    elif s % 2 == 0:
                        nc.vector.tensor_copy(dst, o_p)
                    else:
                        nc.scalar.activation(out=dst, in_=o_p, func=CopyF,
                                             bias=0.0, scale=1.0)
                eng = nc.sync if j % 2 == 0 else nc.gpsimd
                eng.dma_start(
                    out=out_d[:, j * BAND:(j + 1) * BAND], in_=o_band)

    nc.compile()
    return nc


def kernel(x, Wq, bq, Wk, bk, Wv, bv):
    from concourse.bass_utils import run_bass_kernel_spmd

    x = np.ascontiguousarray(np.asarray(x, dtype=np.float32))
    Wq = np.asarray(Wq, dtype=np.float32)
    Wk = np.asarray(Wk, dtype=np.float32)
    Wv = np.asarray(Wv, dtype=np.float32)
    bq = np.asarray(bq, dtype=np.float32)
    bk = np.asarray(bk, dtype=np.float32)
    bv = np.asarray(bv, dtype=np.float32)

    with_qk_bias = bool(np.any(bq) or np.any(bk))
    with_v_bias = bool(np.any(bv))

    key = (with_qk_bias, with_v_bias)
    if key not in _CACHE:
        _CACHE[key] = _build_program(with_qk_bias, with_v_bias)
    nc = _CACHE[key]

    # weight layout prep: wT[cin, ab*128 + c] = W[c, cin, a, b]
    wqT = np.ascontiguousarray(Wq.transpose(1, 2, 3, 0).reshape(C, 4 * C))
    wkT = np.ascontiguousarray(Wk.transpose(1, 2, 3, 0).reshape(C, 4 * C))
    wv = np.ascontiguousarray(Wv.reshape(C, C))

    in_maps = []
    for b in range(B):
        m = {
            "x": np.ascontiguousarray(x[b].reshape(C, HW)),
            "wqT": wqT,
            "wkT": wkT,
            "wv": wv,
        }
        if with_qk_bias:
            m["bq"] = np.ascontiguousarray(bq.reshape(C, 1))
            m["bk"] = np.ascontiguousarray(bk.reshape(C, 1))
        if with_v_bias:
            m["bv"] = np.ascontiguousarray(bv.reshape(C, 1))
        in_maps.append(m)

    res = run_bass_kernel_spmd(nc, in_maps, list(range(N_CORES)))
    out = np.stack([res.results[i]["out"] for i in range(N_CORES)])
    return out.reshape(B, C, H, W).astype(np.float32)


# revision 15
# speedup vs baseline: 1.2811x; 1.0091x over previous
"""CAM (channel attention) module kernel for Trainium2, 8-core data-parallel.

Reference computation (per sample b):
    q = conv2d(x, Wq, stride2, 2x2) -> [C, 4096]
    k = conv2d(x, Wk, stride2, 2x2) -> [C, 4096]
    v = conv2d(x, Wv, 1x1)          -> [C, 16384]
    E = q @ k^T                      [C, C]
    att = softmax(rowmax(E) - E)   (== softmin over rows)
    out = att @ v -> [C, H, W]

Kernel strategy (one sample per NeuronCore, B=8 over 8 cores):
  - The softmax here is extremely peaked (energy entries span +-200), so
    energy errors are amplified exponentially: q/k need ~18+ mantissa
    bits, which rules out bf16 and single-pass fp32r (12-bit) for the
    convs. Native fp32 matmul costs 2 half-rate passes (4 cyc/row).
  - Measured on HW: 4-byte moving operands (fp32 AND fp32r) stream at
    2 cyc/row; bf16 streams at 1 cyc/row. So the cheapest precise
    scheme is split-bf16: x = xh + xl with xh = bf16(x), xl =
    bf16(x - xh) (~16-bit combined); same for the conv weights.
    conv = Wh@xh + Wh@xl + Wl@xh: 3 full-rate bf16 passes (3 cyc/row
    vs fp32's 4) with ~6.5e-4 worst-case output impact.
  - conv produces q in [c, n] layout via strided im2col APs from the
    resident xr/xl2 tiles (4 accumulating taps x 3 passes per band),
    then PE-transposes to [n, c] chunks for the energy contraction.
  - energy e = q k^T in native fp32 (exact; N=128 makes fp32r slow
    there anyway), accumulated over 32 chunk matmuls in one PSUM bank.
  - softmax via one DVE row-min + one ScalarE exp (bias=rowmin,
    scale=-1) with fused accumulated row-sum.
  - out = att @ (Wv x + bv) == (att Wv) @ x + (att bv) 1^T: computes
    M^T = Wv^T att^T on PE ([128,128]), splits M the same way, and
    runs out = Mh@xh + Mh@xl + Ml@xh against the resident split-x
    tiles (3 bf16 passes, ~1e-5 error; reuses the conv's xh/xl).
"""

import numpy as np

B, C, H, W = 8, 128, 128, 128
HW = H * W           # 16384
N_CORES = 8
NB = 8               # number of H-bands (16 input rows each)
BAND = HW // NB      # 2048 x columns per band
QN = (H // 2) * (W // 2)  # 4096 conv output positions
QCHUNK = QN // NB    # 512 conv outputs per band

_CACHE = {}


def _build_program(with_qk_bias: bool, with_v_bias: bool):
    import concourse.tile as tile
    from concourse import bacc, mybir
    from concourse.masks import make_identity

    f32 = mybir.dt.float32
    bf16 = mybir.dt.bfloat16
    Ident = mybir.ActivationFunctionType.Identity
    CopyF = mybir.ActivationFunctionType.Copy
    nc = bacc.Bacc(
        "TRN2", target_bir_lowering=False, debug=False, num_devices=N_CORES)

    x_d = nc.declare_dram_parameter("x", [C, HW], f32, isOutput=False)
    wqT_d = nc.declare_dram_parameter("wqT", [C, 4 * C], f32, isOutput=False)
    wkT_d = nc.declare_dram_parameter("wkT", [C, 4 * C], f32, isOutput=False)
    wv_d = nc.declare_dram_parameter("wv", [C, C], f32, isOutput=False)
    if with_qk_bias:
        bq_d = nc.declare_dram_parameter("bq", [C, 1], f32, isOutput=False)
        bk_d = nc.declare_dram_parameter("bk", [C, 1], f32, isOutput=False)
    if with_v_bias:
        bv_d = nc.declare_dram_parameter("bv", [C, 1], f32, isOutput=False)
    out_d = nc.declare_dram_parameter("out", [C, HW], f32, isOutput=True)

    with tile.TileContext(nc) as tc:
        with (
            tc.tile_pool(name="const", bufs=1) as const,
            tc.tile_pool(name="xstage", bufs=3) as xstage,
            tc.tile_pool(name="xrp", bufs=1) as xrp,
            tc.tile_pool(name="xl2p", bufs=3) as xl2p,
            tc.tile_pool(name="qkT", bufs=1) as qkT,
            tc.tile_pool(name="stage", bufs=3) as stage,
            tc.tile_pool(name="oout", bufs=2) as oout,
            tc.tile_pool(name="small", bufs=2) as small,
            tc.tile_pool(name="pacc", bufs=4, space="PSUM") as pacc,
            tc.tile_pool(name="ptp", bufs=2, space="PSUM") as ptp,
            tc.tile_pool(name="psm", bufs=1, space="PSUM") as psm,
        ):
            ident = const.tile([128, 128], f32, tag="ident")
            make_identity(nc, ident)

            # DMA order: x band 0 first (first conv matmul's critical path),
            # then weights, then the rest of x.
            x_sb = []
            xh_sb = [xrp.tile([C, BAND], bf16, tag=f"xh{j}", name=f"xh{j}")
                     for j in range(NB)]
            xl_sb = [xrp.tile([C, BAND], bf16, tag=f"xl{j}", name=f"xl{j}")
                     for j in range(NB)]
            x0 = xstage.tile([C, BAND], f32, tag="x")
            nc.gpsimd.dma_start(out=x0, in_=x_d[:, 0:BAND])
            x_sb.append(x0)
            wqT_sb = const.tile([C, 4 * C], f32, tag="wqT")
            nc.sync.dma_start(out=wqT_sb, in_=wqT_d[:, :])
            wkT_sb = const.tile([C, 4 * C], f32, tag="wkT")
            nc.sync.dma_start(out=wkT_sb, in_=wkT_d[:, :])
            if with_qk_bias:
                bq_sb = const.tile([C, 1], f32, tag="bq")
                nc.sync.dma_start(out=bq_sb, in_=bq_d[:, :])
                bk_sb = const.tile([C, 1], f32, tag="bk")
                nc.sync.dma_start(out=bk_sb, in_=bk_d[:, :])
            for j in range(1, NB):
                t = xstage.tile([C, BAND], f32, tag="x", name=f"x{j}")
                eng = nc.sync if j % 2 == 0 else nc.gpsimd
                eng.dma_start(out=t, in_=x_d[:, j * BAND:(j + 1) * BAND])
                x_sb.append(t)
            wv_sb = const.tile([C, C], f32, tag="wv")
            nc.sync.dma_start(out=wv_sb, in_=wv_d[:, :])
            if with_v_bias:
                bv_sb = const.tile([C, 1], f32, tag="bv")
                nc.sync.dma_start(out=bv_sb, in_=bv_d[:, :])

            # split the conv weights into bf16 hi/lo parts on DVE (tiny)
            def split_w(w_f32, name):
                hi = const.tile([C, 4 * C], bf16, tag=f"{name}hi")
                nc.vector.tensor_copy(hi, w_f32)
                lo = const.tile([C, 4 * C], bf16, tag=f"{name}lo")
                nc.vector.tensor_tensor(
                    out=lo, in0=w_f32, in1=hi,
                    op=mybir.AluOpType.subtract)
                return hi, lo

            wqh, wql = split_w(wqT_sb, "wq")
            wkh, wkl = split_w(wkT_sb, "wk")

            qT = [qkT.tile([128, QCHUNK], f32, tag=f"qT{j}", name=f"qT{j}")
                  for j in range(NB)]
            kT = [qkT.tile([128, QCHUNK], f32, tag=f"kT{j}", name=f"kT{j}")
                  for j in range(NB)]

            def conv_band(j, wh, wl, xr_v, xl_v, bias_sb):
                """12 accumulating matmuls -> PSUM [128, 512] (q for band j),
                returns the psum tile."""
                acc = pacc.tile([128, QCHUNK], f32, tag="acc")
                n_mm = 0
                for ab in range(4):
                    a, bb = ab // 2, ab % 2
                    for lhsT, rhs in (
                        (wh[:, ab * C:(ab + 1) * C], xr_v[:, :, a, :, bb]),
                        (wh[:, ab * C:(ab + 1) * C], xl_v[:, :, a, :, bb]),
                        (wl[:, ab * C:(ab + 1) * C], xr_v[:, :, a, :, bb]),
                    ):
                        nc.tensor.matmul(acc, lhsT=lhsT, rhs=rhs,
                                         start=(n_mm == 0), stop=(n_mm == 11))
                        n_mm += 1
                return acc

            def conv_band_f32(w_f32, x_v):
                """Band-0 conv straight from the f32 x tile: starts as soon
                as x band 0 + weights have landed, before any split work."""
                acc = pacc.tile([128, QCHUNK], f32, tag="acc")
                for ab in range(4):
                    a, bb = ab // 2, ab % 2
                    nc.tensor.matmul(
                        acc, lhsT=w_f32[:, ab * C:(ab + 1) * C],
                        rhs=x_v[:, :, a, :, bb],
                        start=(ab == 0), stop=(ab == 3))
                return acc

            def emit_transposes(j, qc, kc):
                for T_out, src in ((qT[j], qc), (kT[j], kc)):
                    tp = ptp.tile([128, QCHUNK], f32, tag="tp")
                    for t in range(4):
                        nc.tensor.transpose(
                            tp[:, t * 128:(t + 1) * 128],
                            src[:, t * 128:(t + 1) * 128], ident)
                    nc.scalar.activation(out=T_out, in_=tp, func=CopyF,
                                         bias=0.0, scale=1.0)

            # energy accumulator lives across the whole conv phase: energy
            # chunk matmuls are interleaved into the conv stream so their
            # weight loads hide under conv matmuls and the PE never sits in
            # a low-duty phase (which would re-throttle the HAM clock gate).
            E = psm.tile([128, 128], f32, tag="E")
            e_idx = [0]

            def emit_energy(j):
                for t in range(4):
                    nc.tensor.matmul(
                        E,
                        lhsT=qT[j][:, t * 128:(t + 1) * 128],
                        rhs=kT[j][:, t * 128:(t + 1) * 128],
                        start=(e_idx[0] == 0), stop=(e_idx[0] == NB * 4 - 1))
                    e_idx[0] += 1

            def split_band(j):
                xh_t, xl_t = xh_sb[j], xl_sb[j]
                nc.vector.tensor_copy(xh_t, x_sb[j])
                nc.vector.tensor_tensor(
                    out=xl_t, in0=x_sb[j], in1=xh_t,
                    op=mybir.AluOpType.subtract)

            pend = None
            for j in range(NB):
                if j == 0:
                    x_v = x_sb[0][:].rearrange(
                        "p (i a w b) -> p i a w b", i=8, a=2, w=64, b=2)
                    acc_q = conv_band_f32(wqT_sb, x_v)
                    acc_k = conv_band_f32(wkT_sb, x_v)
                else:
                    split_band(j)
                    if j == 1:
                        split_band(0)  # band 0's split only feeds the
                        # output phase; keep it off the startup critical path
                    xh_t, xl_t = xh_sb[j], xl_sb[j]
                    xr_v = xh_t[:].rearrange(
                        "p (i a w b) -> p i a w b", i=8, a=2, w=64, b=2)
                    xl_v = xl_t[:].rearrange(
                        "p (i a w b) -> p i a w b", i=8, a=2, w=64, b=2)
                    acc_q = conv_band(j, wqh, wql, xr_v, xl_v, None)
                    acc_k = conv_band(j, wkh, wkl, xr_v, xl_v, None)

                qc = stage.tile([128, QCHUNK], f32, tag="qchunk")
                kc = stage.tile([128, QCHUNK], f32, tag="kchunk")
                if with_qk_bias:
                    nc.scalar.activation(out=qc, in_=acc_q, func=Ident,
                                         bias=bq_sb[:, 0:1], scale=1.0)
                    nc.scalar.activation(out=kc, in_=acc_k, func=Ident,
                                         bias=bk_sb[:, 0:1], scale=1.0)
                else:
                    nc.scalar.activation(out=qc, in_=acc_q, func=CopyF,
                                         bias=0.0, scale=1.0)
                    nc.scalar.activation(out=kc, in_=acc_k, func=CopyF,
                                         bias=0.0, scale=1.0)
                # transposes + energy one band behind
                if pend is not None:
                    emit_transposes(*pend)
                    emit_energy(pend[0])
                pend = (j, qc, kc)
            emit_transposes(*pend)
            emit_energy(pend[0])

            # keep the PE busy through the softmax serial chain so the HAM
            # clock gate doesn't re-throttle before the output matmuls
            # (results unused; inputs are long since ready)
            for dw in range(10):
                scratch = pacc.tile([128, 256], f32, tag="acc",
                                    name=f"warm{dw}")
                nc.tensor.matmul(
                    scratch, lhsT=wqh[:, 0:128],
                    rhs=xh_sb[0][:, 0:256],
                    start=True, stop=True)

            # softmin over rows: att = exp(rowmin - E) / Z
            mmin = small.tile([128, 1], f32, tag="mmin")
            nc.vector.tensor_reduce(
                out=mmin, in_=E, axis=mybir.AxisListType.X,
                op=mybir.AluOpType.min)
            w_sb = small.tile([128, 128], f32, tag="w")
            zsum = small.tile([128, 1], f32, tag="z")
            nc.scalar.activation(
                out=w_sb, in_=E, func=mybir.ActivationFunctionType.Exp,
                bias=mmin[:, 0:1], scale=-1.0, accum_out=zsum[:, 0:1])
            rz = small.tile([128, 1], f32, tag="rz")
            nc.vector.reciprocal(rz, zsum)
            att = small.tile([128, 128], f32, tag="att")
            nc.vector.tensor_scalar_mul(att, w_sb, rz[:, 0:1])

            attT_p = psm.tile([128, 128], f32, tag="s2")
            nc.tensor.transpose(attT_p, att, ident)
            attT = small.tile([128, 128], f32, tag="attT")
            nc.vector.tensor_copy(attT, attT_p)

            # M^T[c2, c] = sum_d Wv[d, c2] attT[d, c], split into bf16 hi/lo
            MT_p = psm.tile([128, 128], f32, tag="s2")
            nc.tensor.matmul(MT_p, lhsT=wv_sb, rhs=attT, start=True, stop=True)
            Mh = small.tile([128, 128], bf16, tag="Mh")
            nc.vector.tensor_copy(Mh, MT_p)
            Ml = small.tile([128, 128], bf16, tag="Ml")
            nc.vector.tensor_tensor(
                out=Ml, in0=MT_p, in1=Mh, op=mybir.AluOpType.subtract)

            if with_v_bias:
                abv_p = psm.tile([128, 1], f32, tag="s2")
                nc.tensor.matmul(abv_p, lhsT=attT, rhs=bv_sb[:, 0:1],
                                 start=True, stop=True)
                abv = small.tile([128, 1], f32, tag="abv")
                nc.vector.tensor_copy(abv, abv_p)

            # out[c, n] = sum_c2 M[c, c2] x[c2, n] (+ abv[c]) via bf16 split.
            # Stationary-major order within each band: one LDW for Mh across
            # 8 matmuls, one for Ml across 4, with 4 PSUM accumulators in
            # flight.
            out_dma_engines = [nc.sync, nc.gpsimd, nc.scalar]
            for j in range(NB):
                o_band = oout.tile([128, BAND], f32, tag="oband")
                o_ps = [pacc.tile([128, 512], f32, tag="acc",
                                  name=f"ops{j}_{s}")
                        for s in range(4)]
                for s in range(4):
                    nc.tensor.matmul(
                        o_ps[s], lhsT=Mh,
                        rhs=xh_sb[j][:, s * 512:(s + 1) * 512],
                        start=True, stop=False)
                for s in range(4):
                    nc.tensor.matmul(
                        o_ps[s], lhsT=Mh,
                        rhs=xl_sb[j][:, s * 512:(s + 1) * 512],
                        start=False, stop=False)
                for s in range(4):
                    nc.tensor.matmul(
                        o_ps[s], lhsT=Ml,
                        rhs=xh_sb[j][:, s * 512:(s + 1) * 512],
                        start=False, stop=True)
                for s in range(4):
                    dst = o_band[:, s * 512:(s + 1) * 512]
                    if with_v_bias:
                        nc.scalar.activation(
                            out=dst, in_=o_ps[s], func=Ident,
                            bias=abv[:, 0:1], scale=1.0)
                    elif s % 2 == 0:
                        nc.vector.tensor_copy(dst, o_ps[s])
                    else:
                        nc.scalar.activation(out=dst, in_=o_ps[s], func=CopyF,
                                             bias=0.0, scale=1.0)
                out_dma_engines[j % 3].dma_start(
                    out=out_d[:, j * BAND:(j + 1) * BAND], in_=o_band)

    nc.compile()
    return nc


def kernel(x, Wq, bq, Wk, bk, Wv, bv):
    from concourse.bass_utils import run_bass_kernel_spmd

    x = np.ascontiguousarray(np.asarray(x, dtype=np.float32))
    Wq = np.asarray(Wq, dtype=np.float32)
    Wk = np.asarray(Wk, dtype=np.float32)
    Wv = np.asarray(Wv, dtype=np.float32)
    bq = np.asarray(bq, dtype=np.float32)
    bk = np.asarray(bk, dtype=np.float32)
    bv = np.asarray(bv, dtype=np.float32)

    with_qk_bias = bool(np.any(bq) or np.any(bk))
    with_v_bias = bool(np.any(bv))

    key = (with_qk_bias, with_v_bias)
    if key not in _CACHE:
        _CACHE[key] = _build_program(with_qk_bias, with_v_bias)
    nc = _CACHE[key]

    # weight layout prep: wT[cin, ab*128 + c] = W[c, cin, a, b]
    wqT = np.ascontiguousarray(Wq.transpose(1, 2, 3, 0).reshape(C, 4 * C))
    wkT = np.ascontiguousarray(Wk.transpose(1, 2, 3, 0).reshape(C, 4 * C))
    wv = np.ascontiguousarray(Wv.reshape(C, C))

    in_maps = []
    for b in range(B):
        m = {
            "x": np.ascontiguousarray(x[b].reshape(C, HW)),
            "wqT": wqT,
            "wkT": wkT,
            "wv": wv,
        }
        if with_qk_bias:
            m["bq"] = np.ascontiguousarray(bq.reshape(C, 1))
            m["bk"] = np.ascontiguousarray(bk.reshape(C, 1))
        if with_v_bias:
            m["bv"] = np.ascontiguousarray(bv.reshape(C, 1))
        in_maps.append(m)

    res = run_bass_kernel_spmd(nc, in_maps, list(range(N_CORES)))
    out = np.stack([res.results[i]["out"] for i in range(N_CORES)])
    return out.reshape(B, C, H, W).astype(np.float32)


# revision 16
# speedup vs baseline: 1.2829x; 1.0014x over previous
"""CAM (channel attention) module kernel for Trainium2, 8-core data-parallel.

Reference computation (per sample b):
    q = conv2d(x, Wq, stride2, 2x2) -> [C, 4096]
    k = conv2d(x, Wk, stride2, 2x2) -> [C, 4096]
    v = conv2d(x, Wv, 1x1)          -> [C, 16384]
    E = q @ k^T                      [C, C]
    att = softmax(rowmax(E) - E)   (== softmin over rows)
    out = att @ v -> [C, H, W]

Kernel strategy (one sample per NeuronCore, B=8 over 8 cores):
  - The softmax here is extremely peaked (energy entries span +-200), so
    energy errors are amplified exponentially: q/k need ~18+ mantissa
    bits, which rules out bf16 and single-pass fp32r (12-bit) for the
    convs. Native fp32 matmul costs 2 half-rate passes (4 cyc/row).
  - Measured on HW: 4-byte moving operands (fp32 AND fp32r) stream at
    2 cyc/row; bf16 streams at 1 cyc/row. So the cheapest precise
    scheme is split-bf16: x = xh + xl with xh = bf16(x), xl =
    bf16(x - xh) (~16-bit combined); same for the conv weights.
    conv = Wh@xh + Wh@xl + Wl@xh: 3 full-rate bf16 passes (3 cyc/row
    vs fp32's 4) with ~6.5e-4 worst-case output impact.
  - conv produces q in [c, n] layout via strided im2col APs from the
    resident xr/xl2 tiles (4 accumulating taps x 3 passes per band),
    then PE-transposes to [n, c] chunks for the energy contraction.
  - energy e = q k^T in native fp32 (exact; N=128 makes fp32r slow
    there anyway), accumulated over 32 chunk matmuls in one PSUM bank.
  - softmax via one DVE row-min + one ScalarE exp (bias=rowmin,
    scale=-1) with fused accumulated row-sum.
  - out = att @ (Wv x + bv) == (att Wv) @ x + (att bv) 1^T: computes
    M^T = Wv^T att^T on PE ([128,128]), splits M the same way, and
    runs out = Mh@xh + Mh@xl + Ml@xh against the resident split-x
    tiles (3 bf16 passes, ~1e-5 error; reuses the conv's xh/xl).
"""

import numpy as np

B, C, H, W = 8, 128, 128, 128
HW = H * W           # 16384
N_CORES = 8
NB = 8               # number of H-bands (16 input rows each)
BAND = HW // NB      # 2048 x columns per band
QN = (H // 2) * (W // 2)  # 4096 conv output positions
QCHUNK = QN // NB    # 512 conv outputs per band

_CACHE = {}


def _build_program(with_qk_bias: bool, with_v_bias: bool):
    import concourse.tile as tile
    from concourse import bacc, mybir
    from concourse.masks import make_identity

    f32 = mybir.dt.float32
    bf16 = mybir.dt.bfloat16
    Ident = mybir.ActivationFunctionType.Identity
    CopyF = mybir.ActivationFunctionType.Copy
    nc = bacc.Bacc(
        "TRN2", target_bir_lowering=False, debug=False, num_devices=N_CORES)

    x_d = nc.declare_dram_parameter("x", [C, HW], f32, isOutput=False)
    wqT_d = nc.declare_dram_parameter("wqT", [C, 4 * C], f32, isOutput=False)
    wkT_d = nc.declare_dram_parameter("wkT", [C, 4 * C], f32, isOutput=False)
    wv_d = nc.declare_dram_parameter("wv", [C, C], f32, isOutput=False)
    if with_qk_bias:
        bq_d = nc.declare_dram_parameter("bq", [C, 1], f32, isOutput=False)
        bk_d = nc.declare_dram_parameter("bk", [C, 1], f32, isOutput=False)
    if with_v_bias:
        bv_d = nc.declare_dram_parameter("bv", [C, 1], f32, isOutput=False)
    out_d = nc.declare_dram_parameter("out", [C, HW], f32, isOutput=True)

    with tile.TileContext(nc) as tc:
        with (
            tc.tile_pool(name="const", bufs=1) as const,
            tc.tile_pool(name="xstage", bufs=3) as xstage,
            tc.tile_pool(name="xrp", bufs=1) as xrp,
            tc.tile_pool(name="xl2p", bufs=3) as xl2p,
            tc.tile_pool(name="qkT", bufs=1) as qkT,
            tc.tile_pool(name="stage", bufs=3) as stage,
            tc.tile_pool(name="oout", bufs=3) as oout,
            tc.tile_pool(name="small", bufs=2) as small,
            tc.tile_pool(name="pacc", bufs=4, space="PSUM") as pacc,
            tc.tile_pool(name="ptp", bufs=2, space="PSUM") as ptp,
            tc.tile_pool(name="psm", bufs=1, space="PSUM") as psm,
        ):
            ident = const.tile([128, 128], f32, tag="ident")
            make_identity(nc, ident)

            # DMA order: x band 0 first (first conv matmul's critical path),
            # then weights, then the rest of x.
            x_sb = []
            xh_sb = [xrp.tile([C, BAND], bf16, tag=f"xh{j}", name=f"xh{j}")
                     for j in range(NB)]
            xl_sb = [xrp.tile([C, BAND], bf16, tag=f"xl{j}", name=f"xl{j}")
                     for j in range(NB)]
            x0 = xstage.tile([C, BAND], f32, tag="x")
            nc.sync.dma_start(out=x0[:, 0:BAND // 2], in_=x_d[:, 0:BAND // 2])
            nc.gpsimd.dma_start(
                out=x0[:, BAND // 2:BAND], in_=x_d[:, BAND // 2:BAND])
            x_sb.append(x0)
            wqT_sb = const.tile([C, 4 * C], f32, tag="wqT")
            nc.sync.dma_start(out=wqT_sb, in_=wqT_d[:, :])
            wkT_sb = const.tile([C, 4 * C], f32, tag="wkT")
            nc.sync.dma_start(out=wkT_sb, in_=wkT_d[:, :])
            if with_qk_bias:
                bq_sb = const.tile([C, 1], f32, tag="bq")
                nc.sync.dma_start(out=bq_sb, in_=bq_d[:, :])
                bk_sb = const.tile([C, 1], f32, tag="bk")
                nc.sync.dma_start(out=bk_sb, in_=bk_d[:, :])
            for j in range(1, NB):
                t = xstage.tile([C, BAND], f32, tag="x", name=f"x{j}")
                eng = nc.sync if j % 2 == 0 else nc.gpsimd
                eng.dma_start(out=t, in_=x_d[:, j * BAND:(j + 1) * BAND])
                x_sb.append(t)
            wv_sb = const.tile([C, C], f32, tag="wv")
            nc.sync.dma_start(out=wv_sb, in_=wv_d[:, :])
            if with_v_bias:
                bv_sb = const.tile([C, 1], f32, tag="bv")
                nc.sync.dma_start(out=bv_sb, in_=bv_d[:, :])

            # split the conv weights into bf16 hi/lo parts on DVE (tiny)
            def split_w(w_f32, name):
                hi = const.tile([C, 4 * C], bf16, tag=f"{name}hi")
                nc.vector.tensor_copy(hi, w_f32)
                lo = const.tile([C, 4 * C], bf16, tag=f"{name}lo")
                nc.vector.tensor_tensor(
                    out=lo, in0=w_f32, in1=hi,
                    op=mybir.AluOpType.subtract)
                return hi, lo

            wqh, wql = split_w(wqT_sb, "wq")
            wkh, wkl = split_w(wkT_sb, "wk")

            qT = [qkT.tile([128, QCHUNK], f32, tag=f"qT{j}", name=f"qT{j}")
                  for j in range(NB)]
            kT = [qkT.tile([128, QCHUNK], f32, tag=f"kT{j}", name=f"kT{j}")
                  for j in range(NB)]

            def conv_band(j, wh, wl, xr_v, xl_v, bias_sb):
                """12 accumulating matmuls -> PSUM [128, 512] (q for band j),
                returns the psum tile."""
                acc = pacc.tile([128, QCHUNK], f32, tag="acc")
                n_mm = 0
                for ab in range(4):
                    a, bb = ab // 2, ab % 2
                    for lhsT, rhs in (
                        (wh[:, ab * C:(ab + 1) * C], xr_v[:, :, a, :, bb]),
                        (wh[:, ab * C:(ab + 1) * C], xl_v[:, :, a, :, bb]),
                        (wl[:, ab * C:(ab + 1) * C], xr_v[:, :, a, :, bb]),
                    ):
                        nc.tensor.matmul(acc, lhsT=lhsT, rhs=rhs,
                                         start=(n_mm == 0), stop=(n_mm == 11))
                        n_mm += 1
                return acc

            def conv_band_f32(w_f32, x_v):
                """Band-0 conv straight from the f32 x tile: starts as soon
                as x band 0 + weights have landed, before any split work."""
                acc = pacc.tile([128, QCHUNK], f32, tag="acc")
                for ab in range(4):
                    a, bb = ab // 2, ab % 2
                    nc.tensor.matmul(
                        acc, lhsT=w_f32[:, ab * C:(ab + 1) * C],
                        rhs=x_v[:, :, a, :, bb],
                        start=(ab == 0), stop=(ab == 3))
                return acc

            def emit_transposes(j, qc, kc):
                for T_out, src in ((qT[j], qc), (kT[j], kc)):
                    tp = ptp.tile([128, QCHUNK], f32, tag="tp")
                    for t in range(4):
                        nc.tensor.transpose(
                            tp[:, t * 128:(t + 1) * 128],
                            src[:, t * 128:(t + 1) * 128], ident)
                    nc.scalar.activation(out=T_out, in_=tp, func=CopyF,
                                         bias=0.0, scale=1.0)

            # energy accumulator lives across the whole conv phase: energy
            # chunk matmuls are interleaved into the conv stream so their
            # weight loads hide under conv matmuls and the PE never sits in
            # a low-duty phase (which would re-throttle the HAM clock gate).
            E = psm.tile([128, 128], f32, tag="E")
            e_idx = [0]

            def emit_energy(j):
                for t in range(4):
                    nc.tensor.matmul(
                        E,
                        lhsT=qT[j][:, t * 128:(t + 1) * 128],
                        rhs=kT[j][:, t * 128:(t + 1) * 128],
                        start=(e_idx[0] == 0), stop=(e_idx[0] == NB * 4 - 1))
                    e_idx[0] += 1

            def split_band(j):
                xh_t, xl_t = xh_sb[j], xl_sb[j]
                nc.vector.tensor_copy(xh_t, x_sb[j])
                nc.vector.tensor_tensor(
                    out=xl_t, in0=x_sb[j], in1=xh_t,
                    op=mybir.AluOpType.subtract)

            pend = None
            for j in range(NB):
                if j == 0:
                    x_v = x_sb[0][:].rearrange(
                        "p (i a w b) -> p i a w b", i=8, a=2, w=64, b=2)
                    acc_q = conv_band_f32(wqT_sb, x_v)
                    acc_k = conv_band_f32(wkT_sb, x_v)
                else:
                    split_band(j)
                    if j == 1:
                        split_band(0)  # band 0's split only feeds the
                        # output phase; keep it off the startup critical path
                    xh_t, xl_t = xh_sb[j], xl_sb[j]
                    xr_v = xh_t[:].rearrange(
                        "p (i a w b) -> p i a w b", i=8, a=2, w=64, b=2)
                    xl_v = xl_t[:].rearrange(
                        "p (i a w b) -> p i a w b", i=8, a=2, w=64, b=2)
                    acc_q = conv_band(j, wqh, wql, xr_v, xl_v, None)
                    acc_k = conv_band(j, wkh, wkl, xr_v, xl_v, None)

                qc = stage.tile([128, QCHUNK], f32, tag="qchunk")
                kc = stage.tile([128, QCHUNK], f32, tag="kchunk")
                if with_qk_bias:
                    nc.scalar.activation(out=qc, in_=acc_q, func=Ident,
                                         bias=bq_sb[:, 0:1], scale=1.0)
                    nc.scalar.activation(out=kc, in_=acc_k, func=Ident,
                                         bias=bk_sb[:, 0:1], scale=1.0)
                else:
                    nc.scalar.activation(out=qc, in_=acc_q, func=CopyF,
                                         bias=0.0, scale=1.0)
                    nc.scalar.activation(out=kc, in_=acc_k, func=CopyF,
                                         bias=0.0, scale=1.0)
                # transposes + energy one band behind
                if pend is not None:
                    emit_transposes(*pend)
                    emit_energy(pend[0])
                pend = (j, qc, kc)
            emit_transposes(*pend)
            emit_energy(pend[0])

            # keep the PE busy through the softmax serial chain so the HAM
            # clock gate doesn't re-throttle before the output matmuls
            # (results unused; inputs are long since ready)
            for dw in range(28):
                scratch = pacc.tile([128, 256], f32, tag="acc",
                                    name=f"warm{dw}")
                nc.tensor.matmul(
                    scratch, lhsT=wqh[:, 0:128],
                    rhs=xh_sb[0][:, 0:256],
                    start=True, stop=True)

            # softmin over rows: att = exp(rowmin - E) / Z
            mmin = small.tile([128, 1], f32, tag="mmin")
            nc.vector.tensor_reduce(
                out=mmin, in_=E, axis=mybir.AxisListType.X,
                op=mybir.AluOpType.min)
            w_sb = small.tile([128, 128], f32, tag="w")
            zsum = small.tile([128, 1], f32, tag="z")
            nc.scalar.activation(
                out=w_sb, in_=E, func=mybir.ActivationFunctionType.Exp,
                bias=mmin[:, 0:1], scale=-1.0, accum_out=zsum[:, 0:1])
            rz = small.tile([128, 1], f32, tag="rz")
            nc.vector.reciprocal(rz, zsum)
            att = small.tile([128, 128], f32, tag="att")
            nc.vector.tensor_scalar_mul(att, w_sb, rz[:, 0:1])

            attT_p = psm.tile([128, 128], f32, tag="s2")
            nc.tensor.transpose(attT_p, att, ident)
            attT = small.tile([128, 128], f32, tag="attT")
            nc.vector.tensor_copy(attT, attT_p)

            # M^T[c2, c] = sum_d Wv[d, c2] attT[d, c], split into bf16 hi/lo
            MT_p = psm.tile([128, 128], f32, tag="s2")
            nc.tensor.matmul(MT_p, lhsT=wv_sb, rhs=attT, start=True, stop=True)
            Mh = small.tile([128, 128], bf16, tag="Mh")
            nc.vector.tensor_copy(Mh, MT_p)
            Ml = small.tile([128, 128], bf16, tag="Ml")
            nc.vector.tensor_tensor(
                out=Ml, in0=MT_p, in1=Mh, op=mybir.AluOpType.subtract)

            if with_v_bias:
                abv_p = psm.tile([128, 1], f32, tag="s2")
                nc.tensor.matmul(abv_p, lhsT=attT, rhs=bv_sb[:, 0:1],
                                 start=True, stop=True)
                abv = small.tile([128, 1], f32, tag="abv")
                nc.vector.tensor_copy(abv, abv_p)

            # out[c, n] = sum_c2 M[c, c2] x[c2, n] (+ abv[c]) via bf16 split.
            # Stationary-major order within each band: one LDW for Mh across
            # 8 matmuls, one for Ml across 4, with 4 PSUM accumulators in
            # flight.
            out_dma_engines = [nc.sync, nc.gpsimd, nc.scalar]
            for j in range(NB):
                o_band = oout.tile([128, BAND], f32, tag="oband")
                o_ps = [pacc.tile([128, 512], f32, tag="acc",
                                  name=f"ops{j}_{s}")
                        for s in range(4)]
                for s in range(4):
                    nc.tensor.matmul(
                        o_ps[s], lhsT=Mh,
                        rhs=xh_sb[j][:, s * 512:(s + 1) * 512],
                        start=True, stop=False)
                for s in range(4):
                    nc.tensor.matmul(
                        o_ps[s], lhsT=Mh,
                        rhs=xl_sb[j][:, s * 512:(s + 1) * 512],
                        start=False, stop=False)
                for s in range(4):
                    nc.tensor.matmul(
                        o_ps[s], lhsT=Ml,
                        rhs=xh_sb[j][:, s * 512:(s + 1) * 512],
                        start=False, stop=True)
                for s in range(4):
                    dst = o_band[:, s * 512:(s + 1) * 512]
                    if with_v_bias:
                        nc.scalar.activation(
                            out=dst, in_=o_ps[s], func=Ident,
                            bias=abv[:, 0:1], scale=1.0)
                    elif s % 2 == 0:
                        nc.vector.tensor_copy(dst, o_ps[s])
                    else:
                        nc.scalar.activation(out=dst, in_=o_ps[s], func=CopyF,
                                             bias=0.0, scale=1.0)
                for h in range(2):
                    off = j * BAND + h * (BAND // 2)
                    out_dma_engines[(2 * j + h) % 3].dma_start(
                        out=out_d[:, off:off + BAND // 2],
                        in_=o_band[:, h * (BAND // 2):(h + 1) * (BAND // 2)])

    nc.compile()
    return nc


def kernel(x, Wq, bq, Wk, bk, Wv, bv):
    from concourse.bass_utils import run_bass_kernel_spmd

    x = np.ascontiguousarray(np.asarray(x, dtype=np.float32))
    Wq = np.asarray(Wq, dtype=np.float32)
    Wk = np.asarray(Wk, dtype=np.float32)
    Wv = np.asarray(Wv, dtype=np.float32)
    bq = np.asarray(bq, dtype=np.float32)
    bk = np.asarray(bk, dtype=np.float32)
    bv = np.asarray(bv, dtype=np.float32)

    with_qk_bias = bool(np.any(bq) or np.any(bk))
    with_v_bias = bool(np.any(bv))

    key = (with_qk_bias, with_v_bias)
    if key not in _CACHE:
        _CACHE[key] = _build_program(with_qk_bias, with_v_bias)
    nc = _CACHE[key]

    # weight layout prep: wT[cin, ab*128 + c] = W[c, cin, a, b]
    wqT = np.ascontiguousarray(Wq.transpose(1, 2, 3, 0).reshape(C, 4 * C))
    wkT = np.ascontiguousarray(Wk.transpose(1, 2, 3, 0).reshape(C, 4 * C))
    wv = np.ascontiguousarray(Wv.reshape(C, C))

    in_maps = []
    for b in range(B):
        m = {
            "x": np.ascontiguousarray(x[b].reshape(C, HW)),
            "wqT": wqT,
            "wkT": wkT,
            "wv": wv,
        }
        if with_qk_bias:
            m["bq"] = np.ascontiguousarray(bq.reshape(C, 1))
            m["bk"] = np.ascontiguousarray(bk.reshape(C, 1))
        if with_v_bias:
            m["bv"] = np.ascontiguousarray(bv.reshape(C, 1))
        in_maps.append(m)

    res = run_bass_kernel_spmd(nc, in_maps, list(range(N_CORES)))
    out = np.stack([res.results[i]["out"] for i in range(N_CORES)])
    return out.reshape(B, C, H, W).astype(np.float32)
